# revision 7
# baseline (speedup 1.0000x reference)
"""Trainium2 Bass kernel for CausalSelfAttention (QAT fake-quant + low-rank
adapters + RMSNorm + partial RoPE + GQA causal attention).

Sharding: 8 cores = 2 (batch) x 4 (kv-head groups). Core c handles batch
b = c // 4 and kv group g = c % 4: q heads 4g..4g+3, kv head g. Each core
computes a partial out-projection (its y-column slice x Wproj column slice);
the host sums the 4 partials per batch element.

Weights are folded on the host: the per-row int8 fake-quant (computed
exactly in fp32, matching jnp.round) and the low-rank adapters are combined
into single effective matrices M = fq(W).T + A@B, rounded to bf16. This
removes the adapter matmuls, the fake-quant scale multiplies, and the
z-vector plumbing from the device program entirely; the bf16 rounding of M
adds ~0.2% relative error per projection, well inside tolerance.

Softmax skips the running-max: rms-normed q/k bound |scores| <= sqrt(128),
so exp never overflows fp32. Scores are built transposed ([j, i]) so P.T
never needs materializing. Score tiles are computed in pairs into 2-bank
PSUM tiles so one activation instruction exponentiates 1024 columns. The
softmax denominators are formed by summing the exp tiles with a vector/
gpsimd add tree and contracting the single summed tile with an all-ones
matmul (row-broadcast sums), instead of a full ones-matmul chain. Causal
masking multiplies the diagonal 128x128 sub-block by a triangular 0/1 tile;
the fully-masked leading region of diagonal tiles is never computed.
"""

import sys

sys.path.insert(0, '/opt/trn_rl_repo')

from contextlib import ExitStack

import numpy as np

import concourse.bass as bass
import concourse.bacc as bacc
import concourse.tile as tile
from concourse import mybir
from concourse.bass_utils import run_bass_kernel_spmd
from concourse.masks import make_identity

F32 = mybir.dt.float32
BF16 = mybir.dt.bfloat16
AF = mybir.ActivationFunctionType
ALU = mybir.AluOpType

B, S, DIM = 2, 2048, 2048
NH, NKV = 16, 4
HD = 128
RANK = 16
ROPE_DIMS = 64
HALF = ROPE_DIMS // 2  # 32
BASE = 10000.0
EPS = 1.1920929e-7
EPS128 = 128.0 * EPS
SQRT_HD = float(np.sqrt(128.0))

NT = S // 128            # 16 token tiles of 128
NM = S // 512            # 4 token macros of 512
ND = DIM // 128          # 16 contraction chunks
QF = 4 * HD              # 512 q features per core
KF = HD                  # 128 kv features per core
QKF = QF + KF            # 640 q|k fused width for norm/rope batching
KVF = QKF + KF           # 768 q|k|v fused projection width


def _headbc(ap, nheads):
    """View a [128, 32] AP as [128, nheads, 32] with zero head stride."""
    return bass.AP(tensor=ap.tensor, offset=ap.offset,
                   ap=[list(ap.ap[0]), [0, nheads], list(ap.ap[1])])


def build_program():
    nc = bacc.Bacc(None, target_bir_lowering=False)

    xT = nc.declare_dram_parameter("xT", [DIM, S], BF16, isOutput=False)
    wqkv = nc.declare_dram_parameter("wqkv", [DIM, KVF], BF16, isOutput=False)
    wph = nc.declare_dram_parameter("wph", [QF, DIM], BF16, isOutput=False)
    cs = nc.declare_dram_parameter("cs", [S, HALF], F32, isOutput=False)
    sn = nc.declare_dram_parameter("sn", [S, HALF], F32, isOutput=False)
    gn = nc.declare_dram_parameter("gn", [1, 4], F32, isOutput=False)
    outT = nc.declare_dram_parameter("outT", [DIM, S], BF16, isOutput=True)

    with tile.TileContext(nc) as tc:
        cstack = ExitStack()
        const = cstack.enter_context(tc.tile_pool(name="const", bufs=1))

        ident = const.tile([128, 128], BF16)
        make_identity(nc, ident)
        ones_t = const.tile([128, 128], BF16)
        nc.vector.memset(ones_t, 1.0)
        eps_t = const.tile([128, 1], F32)
        nc.vector.memset(eps_t, EPS128)
        # triangular causal mask for diagonal 128x128 sub-blocks: keep i >= j
        tri = const.tile([128, 128], BF16)
        nc.vector.memset(tri, 1.0)
        nc.gpsimd.affine_select(out=tri, in_=tri, compare_op=ALU.is_ge,
                                fill=0.0, base=0, channel_multiplier=-1,
                                pattern=[[1, 128]])

        gainb = const.tile([128, 4], F32)
        gmul = const.tile([128, 5], F32)
        cos_t = const.tile([128, NT, HALF], F32)
        sin_t = const.tile([128, NT, HALF], F32)

        # ---- folded effective weights (host: fq(W).T + A@B, bf16) ----
        wqkv_i = const.tile([128, ND, KVF], BF16)
        wp_i = const.tile([128, QF // 128, DIM], BF16)

        # ---- resident activation tensors ----
        qT = [const.tile([128, S], BF16, name=f"qT{h}", tag=f"qT{h}")
              for h in range(4)]
        kT = const.tile([128, S], BF16)
        vres = const.tile([128, NT, HD], BF16)
        yT = [const.tile([128, S], BF16, name=f"yT{h}", tag=f"yT{h}")
              for h in range(4)]

        # ================= phase B: projections + norm + rope =============
        bstack = ExitStack()
        xstage = bstack.enter_context(tc.tile_pool(name="xstage", bufs=2 * ND + 2))
        bsb = bstack.enter_context(tc.tile_pool(name="bsb", bufs=3))
        ps_qkv = bstack.enter_context(tc.tile_pool(name="ps_qkv", bufs=3, space="PSUM"))
        ps_tp = bstack.enter_context(tc.tile_pool(name="ps_tp", bufs=2, space="PSUM"))

        def closure_a(st):
            """Evict + stats + rope + normalize for one token tile (no PE)."""
            p, tt = st
            qk = bsb.tile([128, QKF], F32, tag="qk", name="qk")
            # v straight out as bf16 (no scale: folded into the weights)
            nc.scalar.activation(out=vres[:, tt, :], in_=p[:, QKF:KVF],
                                 func=AF.Copy)
            nc.scalar.activation(out=qk, in_=p[:, 0:QKF], func=AF.Copy)
            # rms-norm stats straight from PSUM: cols 0..3 q heads, col 4 k
            stats = bsb.tile([128, 5], F32, tag="stats", name="stats")
            sqscr = bsb.tile([128, HD], F32, tag="sqscr", name="sqscr")
            for c in range(5):
                nc.scalar.activation(out=sqscr,
                                     in_=p[:, c * HD:(c + 1) * HD],
                                     func=AF.Square,
                                     accum_out=stats[:, c:c + 1])
            nc.scalar.activation(out=stats, in_=stats, func=AF.Sqrt,
                                 bias=eps_t)
            nc.vector.reciprocal_approx_fast(out=stats, in_=stats)
            nc.vector.tensor_mul(stats, stats, gmul)

            # batched in-place rope over the 5 fused heads
            q5 = qk.rearrange("p (h c) -> p h c", h=5)
            x1 = q5[:, :, 0:HALF]
            x2 = q5[:, :, HALF:ROPE_DIMS]
            cbc = _headbc(cos_t[:, tt, :], 5)
            sbc = _headbc(sin_t[:, tt, :], 5)
            t1 = bsb.tile([128, 5, HALF], F32, tag="t1", name="t1")
            t2 = bsb.tile([128, 5, HALF], F32, tag="t2", name="t2")
            t3 = bsb.tile([128, 5, HALF], F32, tag="t3", name="t3")
            t4 = bsb.tile([128, 5, HALF], F32, tag="t4", name="t4")
            nc.vector.tensor_mul(t1, x1, cbc)
            nc.vector.tensor_mul(t2, x2, sbc)
            nc.vector.tensor_mul(t3, x2, cbc)
            nc.vector.tensor_mul(t4, x1, sbc)
            nc.vector.tensor_add(x1, t1, t2)
            nc.vector.tensor_sub(x2, t3, t4)

            # per-head normalization in one op (stats broadcast on features)
            qkf = bsb.tile([128, QKF], BF16, tag="qkf", name="qkf")
            sbcast = bass.AP(tensor=stats.tensor, offset=stats.offset,
                             ap=[list(stats.ap[0]), list(stats.ap[1]),
                                 [0, HD]])
            nc.vector.tensor_tensor(out=qkf.rearrange("p (h c) -> p h c", h=5),
                                    in0=q5, in1=sbcast, op=ALU.mult)
            return qkf

        def closure_b(st):
            """Transpose the normalized q|k heads into [d, token] layout."""
            qkf, tt = st
            for c in range(5):
                tp = ps_tp.tile([128, 128], BF16, name="tp", tag="tp")
                nc.tensor.transpose(tp, qkf[:, c * HD:(c + 1) * HD], ident)
                dst = kT if c == 4 else qT[c]
                nc.vector.tensor_copy(out=dst[:, tt * 128:(tt + 1) * 128],
                                      in_=tp)

        pend_a = None
        pend_b = None
        for m in range(NM):
            xts = []
            for d in range(ND):
                xf = xstage.tile([128, 512], BF16, tag="xf", name="xf")
                nc.sync.dma_start(out=xf, in_=xT[d * 128:(d + 1) * 128,
                                               m * 512:(m + 1) * 512])
                xts.append(xf)
                # interleave weight loads with the x stream so the first
                # matmuls can start as soon as chunk 0 lands
                if m == 0:
                    nc.scalar.dma_start(out=wqkv_i[:, d, :],
                                        in_=wqkv[d * 128:(d + 1) * 128, :])
                elif m == 1 and d < 8:
                    dd, half = divmod(d, 2)
                    hsl = slice(half * 1024, (half + 1) * 1024)
                    nc.gpsimd.dma_start(out=wp_i[:, dd, hsl],
                                        in_=wph[dd * 128:(dd + 1) * 128, hsl])
            if m == 0:
                nc.sync.dma_start(out=gainb, in_=gn[:, :].to_broadcast([128, 4]))
                nc.vector.tensor_copy(out=gmul[:, 0:4], in_=gainb)
                nc.vector.memset(gmul[:, 4:5], SQRT_HD)
                nc.sync.dma_start(
                    out=cos_t, in_=cs[:, :].rearrange("(a p) d -> p a d", p=128))
                nc.sync.dma_start(
                    out=sin_t, in_=sn[:, :].rearrange("(a p) d -> p a d", p=128))
            for tsub in range(4):
                tt = m * 4 + tsub
                tsl = slice(tsub * 128, (tsub + 1) * 128)
                p = ps_qkv.tile([128, KVF], F32, name="p")
                for d in range(ND):
                    nc.tensor.matmul(p[:, 0:QF], xts[d][:, tsl],
                                     wqkv_i[:, d, 0:QF],
                                     start=(d == 0), stop=(d == ND - 1))
                    nc.tensor.matmul(p[:, QF:KVF], xts[d][:, tsl],
                                     wqkv_i[:, d, QF:KVF],
                                     start=(d == 0), stop=(d == ND - 1),
                                     skip_group_check=True)
                if pend_b is not None:
                    closure_b(pend_b)
                    pend_b = None
                if pend_a is not None:
                    qkf = closure_a(pend_a)
                    pend_b = (qkf, pend_a[1])
                pend_a = (p, tt)
        closure_b(pend_b)
        qkf = closure_a(pend_a)
        closure_b((qkf, pend_a[1]))
        bstack.close()

        # ============ phase C+D: causal attention + out-projection ========
        adstack = ExitStack()
        epool = adstack.enter_context(tc.tile_pool(name="epool", bufs=NT // 2 + 2))
        esump = adstack.enter_context(tc.tile_pool(name="esump", bufs=3))
        csb = adstack.enter_context(tc.tile_pool(name="csb", bufs=3))
        dsb = adstack.enter_context(tc.tile_pool(name="dsb", bufs=6))
        ps_s = adstack.enter_context(tc.tile_pool(name="ps_s", bufs=2, space="PSUM"))
        ps_y = adstack.enter_context(tc.tile_pool(name="ps_y", bufs=2, space="PSUM"))
        ps_o = adstack.enter_context(tc.tile_pool(name="ps_o", bufs=2, space="PSUM"))

        def emit_outproj(m, ocs):
            isl = slice(m * 512, (m + 1) * 512)
            for oc in ocs:
                osl = slice(oc * 128, (oc + 1) * 128)
                po = ps_o.tile([128, 512], F32, name="po", tag="po")
                for fc in range(4):
                    nc.tensor.matmul(po, wp_i[:, fc, osl], yT[fc][:, isl],
                                     start=(fc == 0), stop=(fc == 3))
                osb = dsb.tile([128, 512], BF16, tag="osb")
                if oc % 2 == 0:
                    nc.scalar.activation(out=osb, in_=po, func=AF.Copy)
                else:
                    nc.vector.tensor_copy(out=osb, in_=po)
                nc.sync.dma_start(out=outT[osl, isl], in_=osb)

        for m in range(NM):
            isl = slice(m * 512, (m + 1) * 512)
            nj = 4 * (m + 1)
            offd = list(range(0, 4 * m))
            diag = list(range(4 * m, 4 * m + 4))
            # diagonal blocks first so their exp+mask hides behind the rest
            score_order = diag + offd
            pairs = [(score_order[2 * i], score_order[2 * i + 1])
                     for i in range(nj // 2)]
            for h in range(4):
                # ---- scores (transposed) + exp, two j-tiles per psum ----
                etiles = {}
                for jc0, jc1 in pairs:
                    psp = ps_s.tile([128, 2, 512], F32, name="psp", tag="ps")
                    e = epool.tile([128, 2, 512], BF16, name="e", tag="e")
                    for half_i, jc in enumerate((jc0, jc1)):
                        t = jc - 4 * m
                        lo = 128 * t if jc >= 4 * m else 0
                        nc.tensor.matmul(
                            psp[:, half_i, lo:],
                            kT[:, jc * 128:(jc + 1) * 128],
                            qT[h][:, m * 512 + lo:(m + 1) * 512],
                            start=True, stop=True)
                        etiles[jc] = (e[:, half_i, :], lo)
                    # one activation exponentiates both halves; the
                    # fully-masked [0:lo] columns hold garbage that is
                    # never read downstream, and the common masked prefix
                    # of the pair is skipped entirely
                    clo = min(128 * (jc0 - 4 * m) if jc0 >= 4 * m else 0,
                              128 * (jc1 - 4 * m) if jc1 >= 4 * m else 0)
                    nc.scalar.activation(out=e[:, :, clo:],
                                         in_=psp[:, :, clo:], func=AF.Exp)
                    for jc in (jc0, jc1):
                        if jc >= 4 * m:
                            ejc, lo = etiles[jc]
                            nc.vector.tensor_mul(ejc[:, lo:lo + 128],
                                                 ejc[:, lo:lo + 128], tri)
                # ---- denominator: sum e tiles (vector + gpsimd partial
                # sums), then one all-ones matmul row-broadcasts the sums
                ng = min(len(offd), max(0, (nj - 2) // 3))
                esum = esump.tile([128, 512], BF16, tag="esum", name="esum")
                e0, _ = etiles[diag[0]]
                nc.vector.tensor_copy(out=esum, in_=e0)
                for jc in diag[1:] + offd[ng:]:
                    e_, lo = etiles[jc]
                    nc.vector.tensor_add(esum[:, lo:], esum[:, lo:],
                                         e_[:, lo:])
                if ng:
                    gsum = esump.tile([128, 512], BF16, tag="gsum",
                                      name="gsum")
                    nc.gpsimd.tensor_copy(out=gsum, in_=etiles[offd[0]][0])
                    for jc in offd[1:ng]:
                        nc.gpsimd.tensor_add(gsum, gsum, etiles[jc][0])
                    nc.vector.tensor_add(esum, esum, gsum)
                # ---- attention output: py = V^T e, chain over j tiles ----
                py = ps_y.tile([128, 512], F32, name="py", tag="py")
                chain = [4 * m] + diag[1:] + offd
                for i, jc in enumerate(chain):
                    e_, lo = etiles[jc]
                    if m == 0 and jc == 3:
                        # bracketing matmul of the group must be full width;
                        # zero the masked columns of the last diagonal tile
                        nc.vector.memset(e_[:, 0:lo], 0.0)
                        lo = 0
                    nc.tensor.matmul(py[:, lo:], vres[:, jc, :], e_[:, lo:],
                                     start=(i == 0), stop=(i == nj - 1),
                                     skip_group_check=(0 < i < nj - 1))
                # ---- denominators broadcast + normalize ----
                pden = ps_o.tile([128, 512], F32, name="pden", tag="po")
                nc.tensor.matmul(pden, ones_t, esum, start=True, stop=True)
                inv = csb.tile([128, 512], F32, tag="inv")
                nc.vector.reciprocal_approx_fast(out=inv, in_=pden)
                nc.vector.tensor_mul(yT[h][:, isl], py, inv)
                # interleave the previous macro's out-projection between heads
                if m >= 1:
                    emit_outproj(m - 1, range(4 * h, 4 * (h + 1)))

        emit_outproj(NM - 1, range(ND))
        adstack.close()
        cstack.close()

    nc.finalize()
    return nc


def make_in_maps(x, Wq, Wk, Wv, Wproj, q_gain, q_A, q_B, k_A, k_B, v_A, v_B,
                 proj_A, proj_B):
    """Shard the full inputs into the 8 per-core input maps (host side)."""
    import ml_dtypes
    f32 = np.float32
    bf16 = ml_dtypes.bfloat16
    x, Wq, Wk, Wv, Wproj, q_gain = (np.asarray(a, f32) for a in
                                    (x, Wq, Wk, Wv, Wproj, q_gain))
    q_A, q_B, k_A, k_B, v_A, v_B, proj_A, proj_B = (
        np.asarray(a, f32) for a in (q_A, q_B, k_A, k_B, v_A, v_B,
                                     proj_A, proj_B))

    def fq(w):
        # round-half-even in fp32 == jnp.round on CPU
        s = np.maximum(np.max(np.abs(w), axis=1) / f32(127.0),
                       f32(1.0 / 127.0)).astype(f32)
        return (np.clip(np.round(w / s[:, None]), f32(-127.0), f32(127.0))
                * s[:, None]).astype(f32)

    # folded effective weights: y = x @ (fq(W).T + A@B)
    Mq = (fq(Wq).T + q_A @ q_B).astype(bf16)
    Mk = (fq(Wk).T + k_A @ k_B).astype(bf16)
    Mv = (fq(Wv).T + v_A @ v_B).astype(bf16)
    Mp = (fq(Wproj).T + proj_A @ proj_B).astype(bf16)

    # rope tables in fp32, matching reference.rope_tables
    inv_freq = (f32(1.0) / (f32(BASE) ** (np.arange(0, ROPE_DIMS, 2,
                dtype=f32) / f32(ROPE_DIMS)))).astype(f32)
    t = np.arange(S, dtype=f32)
    freqs = np.outer(t, inv_freq).astype(f32)
    cos = np.cos(freqs).astype(f32)
    sin = np.sin(freqs).astype(f32)

    in_maps = []
    for c in range(8):
        b, g = divmod(c, 4)
        fq0, fq1 = 512 * g, 512 * (g + 1)
        fk0, fk1 = 128 * g, 128 * (g + 1)
        wqkv = np.concatenate([Mq[:, fq0:fq1], Mk[:, fk0:fk1],
                               Mv[:, fk0:fk1]], axis=1)
        in_maps.append({
            "xT": np.ascontiguousarray(x[b].T).astype(bf16),
            "wqkv": np.ascontiguousarray(wqkv),
            "wph": np.ascontiguousarray(Mp[fq0:fq1, :]),
            "cs": cos,
            "sn": sin,
            "gn": np.ascontiguousarray(q_gain[None, 4 * g:4 * (g + 1)]),
        })
    return in_maps


_PROGRAM = None


def kernel(**inputs):
    global _PROGRAM
    if _PROGRAM is None:
        _PROGRAM = build_program()
    in_maps = make_in_maps(**inputs)
    res = run_bass_kernel_spmd(_PROGRAM, in_maps, core_ids=list(range(8)))
    out = np.empty((B, S, DIM), np.float32)
    for b in range(B):
        acc = res.results[4 * b]["outT"].astype(np.float32)
        for g in range(1, 4):
            acc += res.results[4 * b + g]["outT"].astype(np.float32)
        out[b] = acc.T
    return out
